# revision 1
# baseline (speedup 1.0000x reference)
# Trainium2 Bass kernel for EpiFeatureRebuild (two chained EPI-query stages).
#
# Sharding: core k owns w-columns [8k, 8k+8). Stage-1 computes queries
# p in that window for all 320 horizontal EPIs; stage-2 EPIs (a1, p1)
# with p1 in the window are then fully local -> no cross-core traffic.
#
# Per-EPI pipeline (identical math both stages):
#   Z = conv3x3(feat) @ W0 (shift-decomposed, 6 matmuls: channel rows of
#       (di=0, di=1) stacked on 128 partitions + K=64 bottom rows for di=2)
#   H1[:, (a,w)] = relu(Z[:, (iy_a, w)] + R[:, a])  (R folds b0 + rel0*W0[576])
#   H_{l+1} = relu(W_l^T H_l + b_l)  l=1..3, out = W4^T H4 + b4
# bf16 storage / fp32 PSUM accumulate. Stage-1 output is written directly
# into the zero-padded stage-2 conv layout (RESP); one boundary DMA builds
# the row-shifted bottom copy on partitions 64..127.
import numpy as np

C = 64
A = 9
U = 5          # angular size (v or u) of an EPI
HW = 64        # spatial size (w or h)
NCORES = 8
WLOC = 8       # w-columns per core
IY = [0, 0, 1, 1, 2, 3, 3, 4, 4]

# Stage-1 geometry: per-EPI padded block = 1 + 7*10 + 1 = 72 cols.
BW1 = 72
NB1 = 20       # stage-1 batches: 5 u * 4 groups of 16 h
E1 = 16
NPOS1 = 50     # conv output window (flat 11..60), col j = 10v + c

# Stage-2 geometry: per-block padded = 1 + 7*66 + 1 = 464 cols.
BW2 = 464
NB2 = 36       # stage-2 groups
E2 = 2
NPOS2 = 330    # conv window (flat 67..396), col j = 66u + c

WPCOLS = 3232

_CACHE = {}


def _rel0():
    a = np.arange(A, dtype=np.float32)
    c0 = np.float32(-1.0 + 1.0 / A) + np.float32(2.0 / A) * a
    iy = np.array(IY, np.float32)
    qc0 = (np.float32(2.0) * iy + np.float32(1.0)) / np.float32(U) - np.float32(1.0)
    return (c0 - qc0) * np.float32(U)


def _build_nc(repeat=1):
    import concourse.bass as bass
    import concourse.tile as tile
    from concourse import bacc, mybir
    from contextlib import ExitStack

    f32 = mybir.dt.float32
    bf16 = mybir.dt.bfloat16

    nc = bacc.Bacc("TRN2", target_bir_lowering=False, debug=False)
    xs_d = nc.declare_dram_parameter("xs", [NB1, 128, E1, BW1], bf16, isOutput=False)
    wp_d = nc.declare_dram_parameter("wp", [128, WPCOLS], bf16, isOutput=False)
    wf_d = nc.declare_dram_parameter("wf", [128, 25], f32, isOutput=False)
    out_d = nc.declare_dram_parameter("out", [64, A * WLOC, A, HW], f32, isOutput=True)

    Relu = mybir.ActivationFunctionType.Relu
    Ident = mybir.ActivationFunctionType.Identity
    add_op = mybir.AluOpType.add
    max_op = mybir.AluOpType.max

    eng_ctr = [0]

    def epilogue(out_ap, in_ap, bias_ap, relu):
        # alternate ACT / DVE for load balance
        eng_ctr[0] += 1
        if eng_ctr[0] % 2 == 0:
            nc.scalar.activation(out_ap, in_ap, Relu if relu else Ident,
                                 bias=bias_ap)
        elif relu:
            nc.vector.tensor_scalar(out_ap, in_ap, bias_ap, 0.0, add_op, max_op)
        else:
            nc.vector.tensor_scalar(out_ap, in_ap, bias_ap, None, add_op)

    with tile.TileContext(nc) as tc:
        rep_ctx = ExitStack()
        if repeat > 1:
            rep_ctx.enter_context(tc.For_i(0, repeat, 1))
        with (
            tc.tile_pool(name="const", bufs=1) as cpool,
            tc.tile_pool(name="res", bufs=1) as rpool,
            tc.tile_pool(name="xin", bufs=2) as xpool,
            tc.tile_pool(name="hbuf", bufs=6) as hpool,
            tc.tile_pool(name="ostg", bufs=2) as opool,
            tc.tile_pool(name="zps", bufs=2, space="PSUM") as zpool,
            tc.tile_pool(name="hps", bufs=3, space="PSUM") as hps,
            tc.tile_pool(name="l4ps", bufs=1, space="PSUM") as l4ps,
        ):
            # ---- constants: weights (bf16) + biases (fp32) ----
            wp = cpool.tile([128, WPCOLS], bf16)
            nc.sync.dma_start(wp[:], wp_d[:])
            wf = cpool.tile([128, 25], f32)
            nc.sync.dma_start(wf[:], wf_d[:])
            wa = wp[:, 0:768].rearrange("p (d m) -> p d m", d=3)
            wb = wp[:, 768:1536].rearrange("p (d m) -> p d m", d=3)
            wl = [wp[:, 1536 + 512 * i:1536 + 512 * (i + 1)].rearrange(
                "p (k m) -> p k m", k=2) for i in range(3)]
            w4 = wp[:, 3072:3200].rearrange("p (k m) -> p k m", k=2)
            rt = wf[:, 0:18].rearrange("p (k a) -> p k a", k=2)
            bl = wf[:, 18:24].rearrange("p (k l) -> p k l", k=2)
            b4 = wf[0:64, 24:25]

            # ---- stage-2 conv input, padded resident:
            # block(a,w) x 464; top rows on partitions 0..63 written by S1
            # epilogues; bottom (row-shifted) copy built by one DMA.
            resp = rpool.tile([128, A * WLOC, BW2], bf16)
            nc.vector.memset(resp[:], 0.0)

            def hidden_layers(h1, ncols, nchunk, csz):
                hprev = h1
                for li in range(3):
                    hn = hpool.tile([128, 2, ncols], bf16, tag="h")
                    for cc in range(nchunk):
                        for mt in range(2):
                            ps = hps.tile([128, 384], f32, tag="hp")
                            for k in range(2):
                                nc.tensor.matmul(
                                    ps[:, 0:csz],
                                    (wl[li][:, k, mt * 128:(mt + 1) * 128]),
                                    (hprev[:, k, cc * csz:(cc + 1) * csz]),
                                    start=(k == 0), stop=(k == 1))
                            epilogue(
                                hn[:, mt, cc * csz:(cc + 1) * csz],
                                ps[:, 0:csz], bl[:, mt, li:li + 1], True)
                    hprev = hn
                return hprev

            # ================= STAGE 1 =================
            for b in range(NB1):
                u, hg = b // 4, b % 4
                xin = xpool.tile([128, E1, BW1], bf16, tag="xin")
                nc.sync.dma_start(xin[:], xs_d[b])

                h1 = hpool.tile([128, 2, E1 * A * WLOC], bf16, tag="h")
                for mt in range(2):
                    zt = zpool.tile([128, 2, 512], f32, tag="z")
                    for bank in range(2):
                        es = bank * 8
                        zv = zt[:, bank, 0:8 * NPOS1]
                        for dj in range(3):
                            nc.tensor.matmul(
                                zv, (wa[:, dj, mt * 128:(mt + 1) * 128]),
                                (xin[:, es:es + 8, dj:dj + NPOS1]),
                                start=(dj == 0), stop=False)
                        for dj in range(3):
                            nc.tensor.matmul(
                                zv, (wb[64:128, dj, mt * 128:(mt + 1) * 128]),
                                (xin[64:128, es:es + 8, 10 + dj:10 + dj + NPOS1]),
                                start=False, stop=(dj == 2))
                    zg = zt[:, :, 0:8 * NPOS1].rearrange(
                        "p b (e v c) -> p b e v c", e=8, v=U)
                    h1v = h1[:, mt, :].rearrange(
                        "p (bk e a w) -> p bk e a w", bk=2, e=8, a=A)
                    for a in range(A):
                        epilogue(h1v[:, :, :, a, :],
                                 zg[:, :, :, IY[a], 1:1 + WLOC],
                                 rt[:, mt, a:a + 1], True)

                h4 = hidden_layers(h1, E1 * A * WLOC, 3, 384)

                # L4 -> scatter top rows into RESP (chunk = 4 EPIs = 288 cols)
                respv = resp[0:64, :, :].rearrange(
                    "p (a w) x -> p a w x", a=A)
                for cc in range(4):
                    ps = l4ps.tile([64, 288], f32, tag="l4")
                    for k in range(2):
                        nc.tensor.matmul(
                            ps[:], (w4[:, k, :]),
                            (h4[:, k, cc * 288:(cc + 1) * 288]),
                            start=(k == 0), stop=(k == 1))
                    base = 68 + 66 * u + hg * E1 + cc * 4
                    dst = respv[:, :, :, base:base + 4].transpose((0, 3, 1, 2))
                    psv = ps[:].rearrange("p (e a w) -> p e a w", e=4, a=A)
                    epilogue(dst, psv, b4[:, 0:1], False)

            # ---- stage boundary: build row-shifted bottom copy ----
            nc.sync.dma_start(resp[64:128, :, 0:BW2 - 66],
                              resp[0:64, :, 66:BW2])

            # ================= STAGE 2 =================
            for g in range(NB2):
                h1 = hpool.tile([128, 2, E2 * A * HW], bf16, tag="h")
                for mt in range(2):
                    zt = zpool.tile([128, 2, 512], f32, tag="z")
                    for blk in range(2):
                        zv = zt[:, blk, 0:NPOS2]
                        for dj in range(3):
                            nc.tensor.matmul(
                                zv, (wa[:, dj, mt * 128:(mt + 1) * 128]),
                                (resp[:, 2 * g + blk, dj:dj + NPOS2]),
                                start=(dj == 0), stop=False)
                        for dj in range(3):
                            nc.tensor.matmul(
                                zv, (wb[64:128, dj, mt * 128:(mt + 1) * 128]),
                                (resp[64:128, 2 * g + blk,
                                      66 + dj:66 + dj + NPOS2]),
                                start=False, stop=(dj == 2))
                    h1v = h1[:, mt, :].rearrange(
                        "p (bk a h) -> p bk a h", bk=E2, a=A)
                    for a in range(A):
                        epilogue(h1v[:, :, a, :],
                                 zt[:, :, 1 + 66 * IY[a]:1 + 66 * IY[a] + HW],
                                 rt[:, mt, a:a + 1], True)

                h4 = hidden_layers(h1, E2 * A * HW, 3, 384)

                stg = opool.tile([64, E2 * A * HW], f32, tag="ostg")
                for cc in range(4):
                    ps = l4ps.tile([64, 288], f32, tag="l4")
                    for k in range(2):
                        nc.tensor.matmul(
                            ps[:], (w4[:, k, :]),
                            (h4[:, k, cc * 288:(cc + 1) * 288]),
                            start=(k == 0), stop=(k == 1))
                    epilogue(stg[:, cc * 288:(cc + 1) * 288], ps[:],
                             b4[:, 0:1], False)
                nc.sync.dma_start(
                    out_d[:, 2 * g:2 * g + 2, :, :],
                    stg[:].rearrange("p (b a h) -> p b a h", b=E2, a=A))
        rep_ctx.close()
    nc.compile()
    return nc


def get_nc(repeat=1):
    key = f"nc{repeat}"
    if key not in _CACHE:
        _CACHE[key] = _build_nc(repeat)
    return _CACHE[key]


def host_prep(x, ws, bs):
    """Returns in_maps: list of 8 dicts of numpy arrays."""
    import ml_dtypes
    bfdt = ml_dtypes.bfloat16
    f = np.float32
    W0 = np.asarray(ws[0], f)
    W0r = np.ascontiguousarray(W0[:576].reshape(C, 3, 3, 256))   # [c, di, dj, m]
    wa = np.zeros((128, 3, 256), f)
    wa[:64] = W0r[:, 0]
    wa[64:] = W0r[:, 1]
    wb = np.zeros((128, 3, 256), f)
    wb[64:] = W0r[:, 2]
    rel0 = _rel0()
    R = np.asarray(bs[0], f)[None, :] + rel0[:, None] * W0[576]   # [9, 256]
    rt = np.ascontiguousarray(R.T.reshape(2, 128, A).transpose(1, 0, 2))
    wlt = [np.ascontiguousarray(np.asarray(ws[l], f).reshape(2, 128, 256)
                                .transpose(1, 0, 2)) for l in (1, 2, 3)]
    w4t = np.ascontiguousarray(np.asarray(ws[4], f).reshape(2, 128, 64)
                               .transpose(1, 0, 2))
    blt = np.ascontiguousarray(
        np.stack([np.asarray(bs[l], f).reshape(2, 128) for l in (1, 2, 3)],
                 axis=-1).transpose(1, 0, 2))                     # [128, 2, 3]
    b4t = np.ascontiguousarray(np.asarray(bs[4], f).reshape(64, 1))
    wp = np.zeros((128, WPCOLS), f)
    wp[:, 0:768] = wa.reshape(128, 768)
    wp[:, 768:1536] = wb.reshape(128, 768)
    for i in range(3):
        wp[:, 1536 + 512 * i:1536 + 512 * (i + 1)] = wlt[i].reshape(128, 512)
    wp[:, 3072:3200] = w4t.reshape(128, 128)
    wp = wp.astype(bfdt)
    wf = np.zeros((128, 25), f)
    wf[:, 0:18] = rt.reshape(128, 18)
    wf[:, 18:24] = blt.reshape(128, 6)
    wf[0:64, 24] = b4t[:, 0]

    x0 = np.asarray(x, f)[0]                                      # [C, U, V, H, W]
    xp = np.zeros((C, U, U, HW, HW + 2), f)
    xp[..., 1:HW + 1] = x0
    in_maps = []
    for k in range(NCORES):
        win = xp[..., 8 * k:8 * k + 10]                           # [C,U,V,H,10]
        top = np.zeros((C, U, HW, 7, 10), f)
        top[:, :, :, 1:6, :] = win.transpose(0, 1, 3, 2, 4)       # [C,U,H,V,10]
        blk = np.zeros((C, U, HW, BW1), f)
        blk[..., 1:71] = top.reshape(C, U, HW, 70)
        bot = np.zeros_like(blk)
        bot[..., 0:62] = blk[..., 10:72]
        full = np.concatenate([blk, bot], axis=0)                 # [128,U,H,72]
        xs = np.ascontiguousarray(
            full.transpose(1, 2, 0, 3).reshape(U, 4, E1, 128, BW1)
            .transpose(0, 1, 3, 2, 4).reshape(NB1, 128, E1, BW1))
        in_maps.append({"xs": xs.astype(bfdt), "wp": wp, "wf": wf})
    return in_maps


def assemble(results):
    out = np.empty((1, C, A, A, HW, HW), np.float32)
    for k in range(NCORES):
        ok = np.asarray(results[k]["out"]).reshape(C, A, WLOC, A, HW)
        out[0, :, :, :, :, 8 * k:8 * k + 8] = ok.transpose(0, 3, 1, 4, 2)
    return out


def run(in_maps, trace=False, repeat=1, **kw):
    from concourse.bass_utils import run_bass_kernel_spmd
    return run_bass_kernel_spmd(get_nc(repeat), in_maps, list(range(NCORES)),
                                trace=trace, **kw)


def kernel(x, w0, b0, w1, b1, w2, b2, w3, b3, w4, b4, patchsize=64, ang_factor=9):
    ws = [w0, w1, w2, w3, w4]
    bs = [b0, b1, b2, b3, b4]
    in_maps = host_prep(x, ws, bs)
    res = run(in_maps)
    return assemble(res.results)



# revision 30
# speedup vs baseline: 1.0309x; 1.0309x over previous
# Trainium2 Bass kernel for EpiFeatureRebuild (two chained EPI-query stages).
#
# Sharding: core k owns w-columns [8k, 8k+8). Stage-1 computes queries
# p in that window for all 320 horizontal EPIs; stage-2 EPIs (a1, p1)
# with p1 in the window are then fully local -> no cross-core traffic.
#
# Per-EPI pipeline (identical math both stages):
#   Z = conv3x3(feat) @ W0 (shift-decomposed, 6 matmuls: channel rows of
#       (di=0, di=1) stacked on 128 partitions + K=64 bottom rows for di=2)
#   H1[:, (a,w)] = relu(Z[:, (iy_a, w)] + R[:, a])  (R folds b0 + rel0*W0[576])
#   H_{l+1} = relu(W_l^T H_l + b_l)  l=1..3, out = W4^T H4 + b4
# bf16 storage / fp32 PSUM accumulate. Stage-1 output is written directly
# into the zero-padded stage-2 conv layout (RESP); one boundary DMA builds
# the row-shifted bottom copy on partitions 64..127.
import numpy as np

C = 64
A = 9
U = 5          # angular size (v or u) of an EPI
HW = 64        # spatial size (w or h)
NCORES = 8
WLOC = 8       # w-columns per core
IY = [0, 0, 1, 1, 2, 3, 3, 4, 4]

# Stage-1 geometry: per-EPI padded block = 1 + 7*10 + 1 = 72 cols.
BW1 = 72
NB1 = 20       # stage-1 batches: 5 u * 4 groups of 16 h
E1 = 16
NPOS1 = 50     # conv output window (flat 11..60), col j = 10v + c

# Stage-2 geometry: per-block padded = 1 + 7*66 + 1 = 464 cols.
BW2 = 464
NB2 = 36       # stage-2 groups
E2 = 2
NPOS2 = 330    # conv window (flat 67..396), col j = 66u + c

WPCOLS = 3232

_CACHE = {}


def _rel0():
    a = np.arange(A, dtype=np.float32)
    c0 = np.float32(-1.0 + 1.0 / A) + np.float32(2.0 / A) * a
    iy = np.array(IY, np.float32)
    qc0 = (np.float32(2.0) * iy + np.float32(1.0)) / np.float32(U) - np.float32(1.0)
    return (c0 - qc0) * np.float32(U)


def _build_nc(repeat=1):
    import concourse.bass as bass
    import concourse.tile as tile
    from concourse import bacc, mybir
    from contextlib import ExitStack

    f32 = mybir.dt.float32
    bf16 = mybir.dt.bfloat16

    nc = bacc.Bacc("TRN2", target_bir_lowering=False, debug=False)
    xs_d = nc.declare_dram_parameter("xs", [NB1, 128, E1, BW1], bf16, isOutput=False)
    wp_d = nc.declare_dram_parameter("wp", [128, WPCOLS], bf16, isOutput=False)
    wf_d = nc.declare_dram_parameter("wf", [128, 25], f32, isOutput=False)
    out_d = nc.declare_dram_parameter("out", [64, A, A * WLOC, HW], f32, isOutput=True)

    Relu = mybir.ActivationFunctionType.Relu
    Ident = mybir.ActivationFunctionType.Identity
    add_op = mybir.AluOpType.add
    max_op = mybir.AluOpType.max

    eng_ctr = [0]

    def epilogue(out_ap, in_ap, bias_ap, relu):
        # alternate ACT / DVE for load balance (Pool's Q7 software
        # tensor-ops are ~10x slower per element -- not worth it)
        eng_ctr[0] += 1
        if eng_ctr[0] % 2 == 0:
            nc.scalar.activation(out_ap, in_ap, Relu if relu else Ident,
                                 bias=bias_ap)
        elif relu:
            nc.vector.tensor_scalar(out_ap, in_ap, bias_ap, 0.0, add_op, max_op)
        else:
            nc.vector.tensor_scalar(out_ap, in_ap, bias_ap, None, add_op)

    with tile.TileContext(nc) as tc:
        rep_ctx = ExitStack()
        if repeat > 1:
            rep_ctx.enter_context(tc.For_i(0, repeat, 1))
        with (
            tc.tile_pool(name="const", bufs=1) as cpool,
            tc.tile_pool(name="res", bufs=1) as rpool,
            tc.tile_pool(name="xin", bufs=2) as xpool,
            tc.tile_pool(name="hbuf", bufs=6) as hpool,
            tc.tile_pool(name="ostg", bufs=2) as opool,
            tc.tile_pool(name="zps", bufs=2, space="PSUM") as zpool,
            tc.tile_pool(name="hps", bufs=4, space="PSUM") as hps,
        ):
            # ---- constants: weights (bf16) + biases (fp32) ----
            wp = cpool.tile([128, WPCOLS], bf16)
            nc.sync.dma_start(wp[:], wp_d[:])
            wf = cpool.tile([128, 25], f32)
            wa = wp[:, 0:768].rearrange("p (d m) -> p d m", d=3)
            wb = wp[:, 768:1536].rearrange("p (d m) -> p d m", d=3)
            wl = [wp[:, 1536 + 512 * i:1536 + 512 * (i + 1)].rearrange(
                "p (k m) -> p k m", k=2) for i in range(3)]
            w4 = wp[:, 3072:3200].rearrange("p (k m) -> p k m", k=2)
            rt = wf[:, 0:18].rearrange("p (k a) -> p k a", k=2)
            bl = wf[:, 18:24].rearrange("p (k l) -> p k l", k=2)
            b4 = wf[0:64, 24:25]

            # ---- stage-2 conv input, padded resident:
            # block(a,w) x 464; top rows on partitions 0..63 written by S1
            # epilogues; bottom (row-shifted) copy built by banded DMAs.
            # Only pad columns need zeroing: u-band u occupies cols
            # [68+66u, 132+66u); unwritten = [0,68) + 2-col inter-band gaps
            # + [396,464).
            # L4 PSUM comes from the shared hps ring (partitions 0:64)

            def l4_psum():
                ps = hps.tile([128, 512], f32, tag="hp")
                return ps[0:64]

            resp = rpool.tile([128, A * WLOC, BW2], bf16)
            nc.vector.memset(resp[0:64, 0:36, 0:68], 0.0)
            nc.gpsimd.memset(resp[0:64, 36:72, 0:68], 0.0)
            nc.gpsimd.memset(resp[0:64, :, 396:BW2], 0.0)
            gaps = resp[0:64, :, 132:396].rearrange(
                "p b (u c) -> p b u c", c=66)[:, :, :, 0:2]
            nc.vector.memset(gaps, 0.0)
            def hidden_layers(h1, ncols, mid_cb=None):
                # 512-col chunks (PSUM bank limit), a-block aligned.
                # k-split emission: all k=0 matmuls of a mt first, then
                # k=1 + epilogue -- extra shadow for the producer
                # epilogues of h_prev[:, 1, *].
                bounds = [(c0, min(c0 + 512, ncols))
                          for c0 in range(0, ncols, 512)]
                hprev = h1
                for li in range(3):
                    hn = hpool.tile([128, 2, ncols], bf16, tag="h")
                    for mt in range(2):
                        pss = []
                        for c0, c1 in bounds:
                            ps = hps.tile([128, 512], f32, tag="hp")
                            nc.tensor.matmul(
                                ps[:, 0:c1 - c0],
                                (wl[li][:, 0, mt * 128:(mt + 1) * 128]),
                                (hprev[:, 0, c0:c1]),
                                start=True, stop=False)
                            pss.append(ps)
                        for (c0, c1), ps in zip(bounds, pss):
                            nc.tensor.matmul(
                                ps[:, 0:c1 - c0],
                                (wl[li][:, 1, mt * 128:(mt + 1) * 128]),
                                (hprev[:, 1, c0:c1]),
                                start=False, stop=True)
                            epilogue(
                                hn[:, mt, c0:c1],
                                ps[:, 0:c1 - c0], bl[:, mt, li:li + 1], True)
                    if li == 0 and mid_cb is not None:
                        mid_cb()
                        mid_cb = None
                    hprev = hn
                return hprev

            # ================= STAGE 1 =================
            respv = resp[0:64, :, :].rearrange(
                "p (a w) x -> p a w x", a=A)

            def l4_stage1(h4, u, hg):
                # L4 -> scatter top rows into RESP (a-aligned chunks)
                base = 68 + 66 * u + hg * E1
                for a0, a1 in ((0, 4), (4, 8), (8, 9)):
                    na = a1 - a0
                    ps = l4_psum()
                    for k in range(2):
                        nc.tensor.matmul(
                            ps[:, 0:na * 128], (w4[:, k, :]),
                            (h4[:, k, a0 * 128:a1 * 128]),
                            start=(k == 0), stop=(k == 1))
                    dst = respv[:, a0:a1, :, base:base + E1].transpose(
                        (0, 1, 3, 2))
                    psv = ps[:, 0:na * 128].rearrange(
                        "p (aa e w) -> p aa e w", aa=na, e=E1)
                    epilogue(dst, psv, b4[:, 0:1], False)

            def band_dma(u):
                # u-band of resp top complete: stream its row-shifted
                # bottom copy. bot[:, c] = top[:, c+66].
                c0 = 66 * u
                nc.sync.dma_start(resp[64:128, :, c0:c0 + 66],
                                  resp[0:64, :, c0 + 66:c0 + 132])

            pend = None
            for b in range(NB1):
                u, hg = b // 4, b % 4
                xin = xpool.tile([128, E1, BW1], bf16, tag="xin")
                nc.sync.dma_start(
                    xin[:].rearrange("p e w -> p (e w)"),
                    xs_d[b].rearrange("p e w -> p (e w)"))
                if b == 0:
                    nc.sync.dma_start(wf[:], wf_d[:])
                if b == 1:
                    # tail of the bottom copy: only needs the zeroed pad;
                    # issued here to keep it off the t=0 critical path
                    nc.sync.dma_start(resp[64:128, :, 330:BW2 - 66],
                                      resp[0:64, :, 396:BW2])

                h1 = hpool.tile([128, 2, E1 * A * WLOC], bf16, tag="h")
                for mt in range(2):
                    zt = zpool.tile([128, 2, 512], f32, tag="z")
                    for bank in range(2):
                        es = bank * 8
                        zv = zt[:, bank, 0:8 * NPOS1]
                        for dj in range(3):
                            nc.tensor.matmul(
                                zv, (wa[:, dj, mt * 128:(mt + 1) * 128]),
                                (xin[:, es:es + 8, dj:dj + NPOS1]),
                                start=(dj == 0), stop=False)
                        for dj in range(3):
                            nc.tensor.matmul(
                                zv, (wb[64:128, dj, mt * 128:(mt + 1) * 128]),
                                (xin[64:128, es:es + 8, 10 + dj:10 + dj + NPOS1]),
                                start=False, stop=(dj == 2))
                    zg = zt[:, :, 0:8 * NPOS1].rearrange(
                        "p b (e v c) -> p b e v c", e=8, v=U)
                    # h1 cols a-major: each a-op writes one contiguous
                    # 128-col block -> downstream 512-chunks depend on
                    # exactly 4 a-ops
                    h1v = h1[:, mt, :].rearrange(
                        "p (a bk e w) -> p a bk e w", a=A, bk=2, e=8)
                    for a in range(A):
                        epilogue(h1v[:, a],
                                 zg[:, :, :, IY[a], 1:1 + WLOC],
                                 rt[:, mt, a:a + 1], True)
                if pend is not None:
                    # software-pipelined L4 of the previous batch: runs
                    # in the shadow after this batch's conv
                    l4_stage1(*pend)
                    if pend[2] == 3:
                        band_dma(pend[1])
                    pend = None

                h4 = hidden_layers(h1, E1 * A * WLOC)
                pend = (h4, u, hg)

            l4_stage1(*pend)
            band_dma(4)

            # ================= STAGE 2 =================
            def l4_stage2(h4, g):
                stg = opool.tile([64, E2 * A * HW], f32, tag="ostg")
                for c0 in range(0, E2 * A * HW, 512):
                    c1 = min(c0 + 512, E2 * A * HW)
                    ps = l4_psum()
                    for k in range(2):
                        nc.tensor.matmul(
                            ps[:, 0:c1 - c0], (w4[:, k, :]),
                            (h4[:, k, c0:c1]),
                            start=(k == 0), stop=(k == 1))
                    epilogue(stg[:, c0:c1], ps[:, 0:c1 - c0],
                             b4[:, 0:1], False)
                nc.sync.dma_start(
                    out_d[:, :, 2 * g:2 * g + 2, :],
                    stg[:].rearrange("p (a b h) -> p a b h", a=A, b=E2))

            pend = None
            for g in range(NB2):
                h1 = hpool.tile([128, 2, E2 * A * HW], bf16, tag="h")
                for mt in range(2):
                    zt = zpool.tile([128, 2, 512], f32, tag="z")
                    for blk in range(2):
                        zv = zt[:, blk, 0:NPOS2]
                        for dj in range(3):
                            nc.tensor.matmul(
                                zv, (wa[:, dj, mt * 128:(mt + 1) * 128]),
                                (resp[:, 2 * g + blk, dj:dj + NPOS2]),
                                start=(dj == 0), stop=False)
                        for dj in range(3):
                            nc.tensor.matmul(
                                zv, (wb[64:128, dj, mt * 128:(mt + 1) * 128]),
                                (resp[64:128, 2 * g + blk,
                                      66 + dj:66 + dj + NPOS2]),
                                start=False, stop=(dj == 2))
                    h1v = h1[:, mt, :].rearrange(
                        "p (a bk h) -> p a bk h", a=A, bk=E2)
                    for a in range(A):
                        epilogue(h1v[:, a],
                                 zt[:, :, 1 + 66 * IY[a]:1 + 66 * IY[a] + HW],
                                 rt[:, mt, a:a + 1], True)
                if pend is not None:
                    l4_stage2(*pend)
                    pend = None

                h4 = hidden_layers(h1, E2 * A * HW)
                pend = (h4, g)

            l4_stage2(*pend)
        rep_ctx.close()
    nc.compile()
    return nc


def get_nc(repeat=1):
    key = f"nc{repeat}"
    if key not in _CACHE:
        _CACHE[key] = _build_nc(repeat)
    return _CACHE[key]


def host_prep(x, ws, bs):
    """Returns in_maps: list of 8 dicts of numpy arrays."""
    import ml_dtypes
    bfdt = ml_dtypes.bfloat16
    f = np.float32
    W0 = np.asarray(ws[0], f)
    W0r = np.ascontiguousarray(W0[:576].reshape(C, 3, 3, 256))   # [c, di, dj, m]
    wa = np.zeros((128, 3, 256), f)
    wa[:64] = W0r[:, 0]
    wa[64:] = W0r[:, 1]
    wb = np.zeros((128, 3, 256), f)
    wb[64:] = W0r[:, 2]
    rel0 = _rel0()
    R = np.asarray(bs[0], f)[None, :] + rel0[:, None] * W0[576]   # [9, 256]
    rt = np.ascontiguousarray(R.T.reshape(2, 128, A).transpose(1, 0, 2))
    wlt = [np.ascontiguousarray(np.asarray(ws[l], f).reshape(2, 128, 256)
                                .transpose(1, 0, 2)) for l in (1, 2, 3)]
    w4t = np.ascontiguousarray(np.asarray(ws[4], f).reshape(2, 128, 64)
                               .transpose(1, 0, 2))
    blt = np.ascontiguousarray(
        np.stack([np.asarray(bs[l], f).reshape(2, 128) for l in (1, 2, 3)],
                 axis=-1).transpose(1, 0, 2))                     # [128, 2, 3]
    b4t = np.ascontiguousarray(np.asarray(bs[4], f).reshape(64, 1))
    wp = np.zeros((128, WPCOLS), f)
    wp[:, 0:768] = wa.reshape(128, 768)
    wp[:, 768:1536] = wb.reshape(128, 768)
    for i in range(3):
        wp[:, 1536 + 512 * i:1536 + 512 * (i + 1)] = wlt[i].reshape(128, 512)
    wp[:, 3072:3200] = w4t.reshape(128, 128)
    wp = wp.astype(bfdt)
    wf = np.zeros((128, 25), f)
    wf[:, 0:18] = rt.reshape(128, 18)
    wf[:, 18:24] = blt.reshape(128, 6)
    wf[0:64, 24] = b4t[:, 0]

    x0 = np.asarray(x, f)[0]                                      # [C, U, V, H, W]
    xp = np.zeros((C, U, U, HW, HW + 2), f)
    xp[..., 1:HW + 1] = x0
    in_maps = []
    for k in range(NCORES):
        win = xp[..., 8 * k:8 * k + 10]                           # [C,U,V,H,10]
        top = np.zeros((C, U, HW, 7, 10), f)
        top[:, :, :, 1:6, :] = win.transpose(0, 1, 3, 2, 4)       # [C,U,H,V,10]
        blk = np.zeros((C, U, HW, BW1), f)
        blk[..., 1:71] = top.reshape(C, U, HW, 70)
        bot = np.zeros_like(blk)
        bot[..., 0:62] = blk[..., 10:72]
        full = np.concatenate([blk, bot], axis=0)                 # [128,U,H,72]
        xs = np.ascontiguousarray(
            full.transpose(1, 2, 0, 3).reshape(U, 4, E1, 128, BW1)
            .transpose(0, 1, 3, 2, 4).reshape(NB1, 128, E1, BW1))
        in_maps.append({"xs": xs.astype(bfdt), "wp": wp, "wf": wf})
    return in_maps


def assemble(results):
    out = np.empty((1, C, A, A, HW, HW), np.float32)
    for k in range(NCORES):
        ok = np.asarray(results[k]["out"]).reshape(C, A, A, WLOC, HW)
        out[0, :, :, :, :, 8 * k:8 * k + 8] = ok.transpose(0, 1, 2, 4, 3)
    return out


def run(in_maps, trace=False, repeat=1, **kw):
    from concourse.bass_utils import run_bass_kernel_spmd
    return run_bass_kernel_spmd(get_nc(repeat), in_maps, list(range(NCORES)),
                                trace=trace, **kw)


def kernel(x, w0, b0, w1, b1, w2, b2, w3, b3, w4, b4, patchsize=64, ang_factor=9):
    ws = [w0, w1, w2, w3, w4]
    bs = [b0, b1, b2, b3, b4]
    in_maps = host_prep(x, ws, bs)
    res = run(in_maps)
    return assemble(res.results)



# revision 36
# speedup vs baseline: 1.0545x; 1.0229x over previous
# Trainium2 Bass kernel for EpiFeatureRebuild (two chained EPI-query stages).
#
# Sharding: core k owns w-columns [8k, 8k+8). Stage-1 computes queries
# p in that window for all 320 horizontal EPIs; stage-2 EPIs (a1, p1)
# with p1 in the window are then fully local -> no cross-core traffic.
#
# Per-EPI pipeline (identical math both stages):
#   Z = conv3x3(feat) @ W0 (shift-decomposed, 6 matmuls: channel rows of
#       (di=0, di=1) stacked on 128 partitions + K=64 bottom rows for di=2)
#   H1[:, (a,w)] = relu(Z[:, (iy_a, w)] + R[:, a])  (R folds b0 + rel0*W0[576])
#   H_{l+1} = relu(W_l^T H_l + b_l)  l=1..3, out = W4^T H4 + b4
# bf16 storage / fp32 PSUM accumulate. Stage-1 output is written directly
# into the zero-padded stage-2 conv layout (RESP); one boundary DMA builds
# the row-shifted bottom copy on partitions 64..127.
import numpy as np

C = 64
A = 9
U = 5          # angular size (v or u) of an EPI
HW = 64        # spatial size (w or h)
NCORES = 8
WLOC = 8       # w-columns per core
IY = [0, 0, 1, 1, 2, 3, 3, 4, 4]

# Stage-1 geometry: per-EPI padded block = 1 + 7*10 + 1 = 72 cols.
BW1 = 72
NB1 = 20       # stage-1 batches: 5 u * 4 groups of 16 h
E1 = 16
NPOS1 = 50     # conv output window (flat 11..60), col j = 10v + c

# Stage-2 geometry: per-block padded = 1 + 7*66 + 1 = 464 cols.
BW2 = 464
NB2 = 36       # stage-2 groups
E2 = 2
NPOS2 = 330    # conv window (flat 67..396), col j = 66u + c

WPCOLS = 3232

_CACHE = {}


def _rel0():
    a = np.arange(A, dtype=np.float32)
    c0 = np.float32(-1.0 + 1.0 / A) + np.float32(2.0 / A) * a
    iy = np.array(IY, np.float32)
    qc0 = (np.float32(2.0) * iy + np.float32(1.0)) / np.float32(U) - np.float32(1.0)
    return (c0 - qc0) * np.float32(U)


def _build_nc(repeat=1):
    import concourse.bass as bass
    import concourse.tile as tile
    from concourse import bacc, mybir
    from contextlib import ExitStack

    f32 = mybir.dt.float32
    bf16 = mybir.dt.bfloat16

    nc = bacc.Bacc("TRN2", target_bir_lowering=False, debug=False)
    xs_d = nc.declare_dram_parameter("xs", [NB1, 128, E1, BW1], bf16, isOutput=False)
    wp_d = nc.declare_dram_parameter("wp", [128, WPCOLS], bf16, isOutput=False)
    wf_d = nc.declare_dram_parameter("wf", [128, 25], f32, isOutput=False)
    out_d = nc.declare_dram_parameter("out", [64, A, A * WLOC, HW], f32, isOutput=True)

    Relu = mybir.ActivationFunctionType.Relu
    Ident = mybir.ActivationFunctionType.Identity
    add_op = mybir.AluOpType.add
    max_op = mybir.AluOpType.max

    eng_ctr = [0]

    def epilogue(out_ap, in_ap, bias_ap, relu, pool=False):
        # alternate ACT / DVE for load balance; Pool (slow Q7 software
        # op) only for explicitly offloaded non-urgent epilogues
        if pool:
            nc.gpsimd.tensor_scalar(out_ap, in_ap, bias_ap, None, add_op)
            return
        eng_ctr[0] += 1
        if eng_ctr[0] % 2 == 0:
            nc.scalar.activation(out_ap, in_ap, Relu if relu else Ident,
                                 bias=bias_ap)
        elif relu:
            nc.vector.tensor_scalar(out_ap, in_ap, bias_ap, 0.0, add_op, max_op)
        else:
            nc.vector.tensor_scalar(out_ap, in_ap, bias_ap, None, add_op)

    with tile.TileContext(nc) as tc:
        rep_ctx = ExitStack()
        if repeat > 1:
            rep_ctx.enter_context(tc.For_i(0, repeat, 1))
        with (
            tc.tile_pool(name="const", bufs=1) as cpool,
            tc.tile_pool(name="res", bufs=1) as rpool,
            tc.tile_pool(name="xin", bufs=2) as xpool,
            tc.tile_pool(name="hbuf", bufs=6) as hpool,
            tc.tile_pool(name="ostg", bufs=2) as opool,
            tc.tile_pool(name="zps", bufs=2, space="PSUM") as zpool,
            tc.tile_pool(name="hps", bufs=4, space="PSUM") as hps,
        ):
            # ---- constants: weights (bf16) + biases (fp32) ----
            wp = cpool.tile([128, WPCOLS], bf16)
            nc.sync.dma_start(wp[:], wp_d[:])
            wf = cpool.tile([128, 25], f32)
            wa = wp[:, 0:768].rearrange("p (d m) -> p d m", d=3)
            wb = wp[:, 768:1536].rearrange("p (d m) -> p d m", d=3)
            wl = [wp[:, 1536 + 512 * i:1536 + 512 * (i + 1)].rearrange(
                "p (k m) -> p k m", k=2) for i in range(3)]
            w4 = wp[:, 3072:3200].rearrange("p (k m) -> p k m", k=2)
            rt = wf[:, 0:18].rearrange("p (k a) -> p k a", k=2)
            bl = wf[:, 18:24].rearrange("p (k l) -> p k l", k=2)
            b4 = wf[0:64, 24:25]

            # ---- stage-2 conv input, padded resident:
            # block(a,w) x 464; top rows on partitions 0..63 written by S1
            # epilogues; bottom (row-shifted) copy built by banded DMAs.
            # Only pad columns need zeroing: u-band u occupies cols
            # [68+66u, 132+66u); unwritten = [0,68) + 2-col inter-band gaps
            # + [396,464).
            # L4 PSUM comes from the shared hps ring (partitions 0:64)

            def l4_psum():
                ps = hps.tile([128, 512], f32, tag="hp")
                return ps[0:64]

            resp = rpool.tile([128, A * WLOC, BW2], bf16)

            def resp_memsets():
                nc.vector.memset(resp[0:64, 0:36, 0:68], 0.0)
                nc.gpsimd.memset(resp[0:64, 36:72, 0:68], 0.0)
                nc.gpsimd.memset(resp[0:64, :, 396:BW2], 0.0)
                gaps = resp[0:64, :, 132:396].rearrange(
                    "p b (u c) -> p b u c", c=66)[:, :, :, 0:2]
                nc.vector.memset(gaps, 0.0)
            def hidden_layers(h1, ncols, mid_cb=None):
                # 512-col chunks (PSUM bank limit), a-block aligned.
                # k-split emission: all k=0 matmuls of a mt first, then
                # k=1 + epilogue -- extra shadow for the producer
                # epilogues of h_prev[:, 1, *].
                bounds = [(c0, min(c0 + 512, ncols))
                          for c0 in range(0, ncols, 512)]
                hprev = h1
                for li in range(3):
                    hn = hpool.tile([128, 2, ncols], bf16, tag="h")
                    for mt in range(2):
                        pss = []
                        for c0, c1 in bounds:
                            ps = hps.tile([128, 512], f32, tag="hp")
                            nc.tensor.matmul(
                                ps[:, 0:c1 - c0],
                                (wl[li][:, 0, mt * 128:(mt + 1) * 128]),
                                (hprev[:, 0, c0:c1]),
                                start=True, stop=False)
                            pss.append(ps)
                        for (c0, c1), ps in zip(bounds, pss):
                            nc.tensor.matmul(
                                ps[:, 0:c1 - c0],
                                (wl[li][:, 1, mt * 128:(mt + 1) * 128]),
                                (hprev[:, 1, c0:c1]),
                                start=False, stop=True)
                            epilogue(
                                hn[:, mt, c0:c1],
                                ps[:, 0:c1 - c0], bl[:, mt, li:li + 1], True)
                    if li == 0 and mid_cb is not None:
                        mid_cb()
                        mid_cb = None
                    hprev = hn
                return hprev

            # ================= STAGE 1 =================
            respv = resp[0:64, :, :].rearrange(
                "p (a w) x -> p a w x", a=A)

            def l4_stage1(h4, u, hg):
                # L4 -> scatter top rows into RESP (a-aligned chunks)
                base = 68 + 66 * u + hg * E1
                for a0, a1 in ((0, 4), (4, 8), (8, 9)):
                    na = a1 - a0
                    ps = l4_psum()
                    for k in range(2):
                        nc.tensor.matmul(
                            ps[:, 0:na * 128], (w4[:, k, :]),
                            (h4[:, k, a0 * 128:a1 * 128]),
                            start=(k == 0), stop=(k == 1))
                    dst = respv[:, a0:a1, :, base:base + E1].transpose(
                        (0, 1, 3, 2))
                    psv = ps[:, 0:na * 128].rearrange(
                        "p (aa e w) -> p aa e w", aa=na, e=E1)
                    epilogue(dst, psv, b4[:, 0:1], False)

            def band_dma(u, split=False):
                # u-band of resp top complete: stream its row-shifted
                # bottom copy. bot[:, c] = top[:, c+66].
                c0 = 66 * u
                blk_bounds = ((0, 8), (8, 72)) if split else ((0, 72),)
                for b0, b1 in blk_bounds:
                    nc.gpsimd.dma_start(
                        resp[64:128, b0:b1, c0:c0 + 66],
                        resp[0:64, b0:b1, c0 + 66:c0 + 132])

            pend = None
            for b in range(NB1):
                u, hg = b // 4, b % 4
                xin = xpool.tile([128, E1, BW1], bf16, tag="xin")
                nc.gpsimd.dma_start(
                    xin[:].rearrange("p e w -> p (e w)"),
                    xs_d[b].rearrange("p e w -> p (e w)"))
                if b == 0:
                    nc.sync.dma_start(wf[:], wf_d[:])
                    resp_memsets()
                if b == 1:
                    # tail of the bottom copy: only needs the zeroed pad;
                    # issued here to keep it off the t=0 critical path
                    nc.sync.dma_start(resp[64:128, :, 330:BW2 - 66],
                                      resp[0:64, :, 396:BW2])

                h1 = hpool.tile([128, 2, E1 * A * WLOC], bf16, tag="h")
                for mt in range(2):
                    zt = zpool.tile([128, 2, 512], f32, tag="z")
                    for bank in range(2):
                        es = bank * 8
                        zv = zt[:, bank, 0:8 * NPOS1]
                        for dj in range(3):
                            nc.tensor.matmul(
                                zv, (wa[:, dj, mt * 128:(mt + 1) * 128]),
                                (xin[:, es:es + 8, dj:dj + NPOS1]),
                                start=(dj == 0), stop=False)
                        for dj in range(3):
                            nc.tensor.matmul(
                                zv, (wb[64:128, dj, mt * 128:(mt + 1) * 128]),
                                (xin[64:128, es:es + 8, 10 + dj:10 + dj + NPOS1]),
                                start=False, stop=(dj == 2))
                    zg = zt[:, :, 0:8 * NPOS1].rearrange(
                        "p b (e v c) -> p b e v c", e=8, v=U)
                    # h1 cols a-major: each a-op writes one contiguous
                    # 128-col block -> downstream 512-chunks depend on
                    # exactly 4 a-ops
                    h1v = h1[:, mt, :].rearrange(
                        "p (a bk e w) -> p a bk e w", a=A, bk=2, e=8)
                    for a in range(A):
                        epilogue(h1v[:, a],
                                 zg[:, :, :, IY[a], 1:1 + WLOC],
                                 rt[:, mt, a:a + 1], True)
                if pend is not None:
                    # software-pipelined L4 of the previous batch: runs
                    # in the shadow after this batch's conv
                    l4_stage1(*pend)
                    if pend[2] == 3:
                        band_dma(pend[1])
                    pend = None

                h4 = hidden_layers(h1, E1 * A * WLOC)
                pend = (h4, u, hg)

            l4_stage1(*pend)
            band_dma(4, split=True)

            # ================= STAGE 2 =================
            def l4_stage2(h4, g):
                stg = opool.tile([64, E2 * A * HW], f32, tag="ostg")
                for c0 in range(0, E2 * A * HW, 512):
                    c1 = min(c0 + 512, E2 * A * HW)
                    ps = l4_psum()
                    for k in range(2):
                        nc.tensor.matmul(
                            ps[:, 0:c1 - c0], (w4[:, k, :]),
                            (h4[:, k, c0:c1]),
                            start=(k == 0), stop=(k == 1))
                    epilogue(stg[:, c0:c1], ps[:, 0:c1 - c0],
                             b4[:, 0:1], False)
                nc.gpsimd.dma_start(
                    out_d[:, :, 2 * g:2 * g + 2, :],
                    stg[:].rearrange("p (a b h) -> p a b h", a=A, b=E2))

            pend = None
            for g in range(NB2):
                h1 = hpool.tile([128, 2, E2 * A * HW], bf16, tag="h")
                for mt in range(2):
                    zt = zpool.tile([128, 2, 512], f32, tag="z")
                    for blk in range(2):
                        zv = zt[:, blk, 0:NPOS2]
                        for dj in range(3):
                            nc.tensor.matmul(
                                zv, (wa[:, dj, mt * 128:(mt + 1) * 128]),
                                (resp[:, 2 * g + blk, dj:dj + NPOS2]),
                                start=(dj == 0), stop=False)
                        for dj in range(3):
                            nc.tensor.matmul(
                                zv, (wb[64:128, dj, mt * 128:(mt + 1) * 128]),
                                (resp[64:128, 2 * g + blk,
                                      66 + dj:66 + dj + NPOS2]),
                                start=False, stop=(dj == 2))
                    h1v = h1[:, mt, :].rearrange(
                        "p (a bk h) -> p a bk h", a=A, bk=E2)
                    for a in range(A):
                        epilogue(h1v[:, a],
                                 zt[:, :, 1 + 66 * IY[a]:1 + 66 * IY[a] + HW],
                                 rt[:, mt, a:a + 1], True)
                if pend is not None:
                    l4_stage2(*pend)
                    pend = None

                h4 = hidden_layers(h1, E2 * A * HW)
                pend = (h4, g)

            l4_stage2(*pend)
        rep_ctx.close()
    nc.compile()
    return nc


def get_nc(repeat=1):
    key = f"nc{repeat}"
    if key not in _CACHE:
        _CACHE[key] = _build_nc(repeat)
    return _CACHE[key]


def host_prep(x, ws, bs):
    """Returns in_maps: list of 8 dicts of numpy arrays."""
    import ml_dtypes
    bfdt = ml_dtypes.bfloat16
    f = np.float32
    W0 = np.asarray(ws[0], f)
    W0r = np.ascontiguousarray(W0[:576].reshape(C, 3, 3, 256))   # [c, di, dj, m]
    wa = np.zeros((128, 3, 256), f)
    wa[:64] = W0r[:, 0]
    wa[64:] = W0r[:, 1]
    wb = np.zeros((128, 3, 256), f)
    wb[64:] = W0r[:, 2]
    rel0 = _rel0()
    R = np.asarray(bs[0], f)[None, :] + rel0[:, None] * W0[576]   # [9, 256]
    rt = np.ascontiguousarray(R.T.reshape(2, 128, A).transpose(1, 0, 2))
    wlt = [np.ascontiguousarray(np.asarray(ws[l], f).reshape(2, 128, 256)
                                .transpose(1, 0, 2)) for l in (1, 2, 3)]
    w4t = np.ascontiguousarray(np.asarray(ws[4], f).reshape(2, 128, 64)
                               .transpose(1, 0, 2))
    blt = np.ascontiguousarray(
        np.stack([np.asarray(bs[l], f).reshape(2, 128) for l in (1, 2, 3)],
                 axis=-1).transpose(1, 0, 2))                     # [128, 2, 3]
    b4t = np.ascontiguousarray(np.asarray(bs[4], f).reshape(64, 1))
    wp = np.zeros((128, WPCOLS), f)
    wp[:, 0:768] = wa.reshape(128, 768)
    wp[:, 768:1536] = wb.reshape(128, 768)
    for i in range(3):
        wp[:, 1536 + 512 * i:1536 + 512 * (i + 1)] = wlt[i].reshape(128, 512)
    wp[:, 3072:3200] = w4t.reshape(128, 128)
    wp = wp.astype(bfdt)
    wf = np.zeros((128, 25), f)
    wf[:, 0:18] = rt.reshape(128, 18)
    wf[:, 18:24] = blt.reshape(128, 6)
    wf[0:64, 24] = b4t[:, 0]

    x0 = np.asarray(x, f)[0]                                      # [C, U, V, H, W]
    xp = np.zeros((C, U, U, HW, HW + 2), f)
    xp[..., 1:HW + 1] = x0
    in_maps = []
    for k in range(NCORES):
        win = xp[..., 8 * k:8 * k + 10]                           # [C,U,V,H,10]
        top = np.zeros((C, U, HW, 7, 10), f)
        top[:, :, :, 1:6, :] = win.transpose(0, 1, 3, 2, 4)       # [C,U,H,V,10]
        blk = np.zeros((C, U, HW, BW1), f)
        blk[..., 1:71] = top.reshape(C, U, HW, 70)
        bot = np.zeros_like(blk)
        bot[..., 0:62] = blk[..., 10:72]
        full = np.concatenate([blk, bot], axis=0)                 # [128,U,H,72]
        xs = np.ascontiguousarray(
            full.transpose(1, 2, 0, 3).reshape(U, 4, E1, 128, BW1)
            .transpose(0, 1, 3, 2, 4).reshape(NB1, 128, E1, BW1))
        in_maps.append({"xs": xs.astype(bfdt), "wp": wp, "wf": wf})
    return in_maps


def assemble(results):
    out = np.empty((1, C, A, A, HW, HW), np.float32)
    for k in range(NCORES):
        ok = np.asarray(results[k]["out"]).reshape(C, A, A, WLOC, HW)
        out[0, :, :, :, :, 8 * k:8 * k + 8] = ok.transpose(0, 1, 2, 4, 3)
    return out


def run(in_maps, trace=False, repeat=1, **kw):
    from concourse.bass_utils import run_bass_kernel_spmd
    return run_bass_kernel_spmd(get_nc(repeat), in_maps, list(range(NCORES)),
                                trace=trace, **kw)


def kernel(x, w0, b0, w1, b1, w2, b2, w3, b3, w4, b4, patchsize=64, ang_factor=9):
    ws = [w0, w1, w2, w3, w4]
    bs = [b0, b1, b2, b3, b4]
    in_maps = host_prep(x, ws, bs)
    res = run(in_maps)
    return assemble(res.results)



# revision 37
# speedup vs baseline: 1.2499x; 1.1852x over previous
# Trainium2 Bass kernel for EpiFeatureRebuild (two chained EPI-query stages).
#
# Sharding: core k owns w-columns [8k, 8k+8). Stage-1 computes queries
# p in that window for all 320 horizontal EPIs; stage-2 EPIs (a1, p1)
# with p1 in the window are then fully local -> no cross-core traffic.
#
# Per-EPI pipeline (identical math both stages):
#   Z = conv3x3(feat) @ W0 (shift-decomposed, 6 matmuls: channel rows of
#       (di=0, di=1) stacked on 128 partitions + K=64 bottom rows for di=2)
#   H1[:, (a,w)] = relu(Z[:, (iy_a, w)] + R[:, a])  (R folds b0 + rel0*W0[576])
#   H_{l+1} = relu(W_l^T H_l + b_l)  l=1..3, out = W4^T H4 + b4
# bf16 storage / fp32 PSUM accumulate. Stage-1 output is written directly
# into the zero-padded stage-2 conv layout (RESP); one boundary DMA builds
# the row-shifted bottom copy on partitions 64..127.
import numpy as np

C = 64
A = 9
U = 5          # angular size (v or u) of an EPI
HW = 64        # spatial size (w or h)
NCORES = 8
WLOC = 8       # w-columns per core
IY = [0, 0, 1, 1, 2, 3, 3, 4, 4]

# Stage-1 geometry: per-EPI padded block = 1 + 7*10 + 1 = 72 cols.
BW1 = 72
NB1 = 20       # stage-1 batches: 5 u * 4 groups of 16 h
E1 = 16
NPOS1 = 50     # conv output window (flat 11..60), col j = 10v + c

# Stage-2 geometry: per-block padded = 1 + 7*66 + 1 = 464 cols.
BW2 = 464
NB2 = 36       # stage-2 groups
E2 = 2
NPOS2 = 330    # conv window (flat 67..396), col j = 66u + c

WPCOLS = 3232

_CACHE = {}


def _rel0():
    a = np.arange(A, dtype=np.float32)
    c0 = np.float32(-1.0 + 1.0 / A) + np.float32(2.0 / A) * a
    iy = np.array(IY, np.float32)
    qc0 = (np.float32(2.0) * iy + np.float32(1.0)) / np.float32(U) - np.float32(1.0)
    return (c0 - qc0) * np.float32(U)


def _build_nc(repeat=1):
    import concourse.bass as bass
    import concourse.tile as tile
    from concourse import bacc, mybir
    from contextlib import ExitStack

    f32 = mybir.dt.float32
    bf16 = mybir.dt.bfloat16

    nc = bacc.Bacc("TRN2", target_bir_lowering=False, debug=False)
    xs_d = nc.declare_dram_parameter("xs", [NB1, 128, E1, BW1], bf16, isOutput=False)
    wp_d = nc.declare_dram_parameter("wp", [128, WPCOLS], bf16, isOutput=False)
    wf_d = nc.declare_dram_parameter("wf", [128, 25], f32, isOutput=False)
    out_d = nc.declare_dram_parameter("out", [64, A, A * WLOC, HW], f32, isOutput=True)

    Relu = mybir.ActivationFunctionType.Relu
    Ident = mybir.ActivationFunctionType.Identity
    add_op = mybir.AluOpType.add
    max_op = mybir.AluOpType.max

    eng_ctr = [0]

    def epilogue(out_ap, in_ap, bias_ap, relu, pool=False):
        # alternate ACT / DVE for load balance; Pool (slow Q7 software
        # op) only for explicitly offloaded non-urgent epilogues
        if pool:
            nc.gpsimd.tensor_scalar(out_ap, in_ap, bias_ap, None, add_op)
            return
        eng_ctr[0] += 1
        if eng_ctr[0] % 2 == 0:
            nc.scalar.activation(out_ap, in_ap, Relu if relu else Ident,
                                 bias=bias_ap)
        elif relu:
            nc.vector.tensor_scalar(out_ap, in_ap, bias_ap, 0.0, add_op, max_op)
        else:
            nc.vector.tensor_scalar(out_ap, in_ap, bias_ap, None, add_op)

    with tile.TileContext(nc) as tc:
        rep_ctx = ExitStack()
        if repeat > 1:
            rep_ctx.enter_context(tc.For_i(0, repeat, 1))
        with (
            tc.tile_pool(name="const", bufs=1) as cpool,
            tc.tile_pool(name="res", bufs=1) as rpool,
            tc.tile_pool(name="xin", bufs=2) as xpool,
            tc.tile_pool(name="hbuf", bufs=6) as hpool,
            tc.tile_pool(name="ostg", bufs=2) as opool,
            tc.tile_pool(name="zps", bufs=2, space="PSUM") as zpool,
            tc.tile_pool(name="hps", bufs=4, space="PSUM") as hps,
        ):
            # ---- constants: weights (bf16) + biases (fp32) ----
            wp = cpool.tile([128, WPCOLS], bf16)
            # conv weights (wa/wb) first: the first conv only needs cols
            # 0:1536, so it can start while the MLP weights stream in
            nc.sync.dma_start(wp[:, 0:1536], wp_d[:, 0:1536])
            nc.sync.dma_start(wp[:, 1536:WPCOLS], wp_d[:, 1536:WPCOLS])
            wf = cpool.tile([128, 25], f32)
            wa = wp[:, 0:768].rearrange("p (d m) -> p d m", d=3)
            wb = wp[:, 768:1536].rearrange("p (d m) -> p d m", d=3)
            wl = [wp[:, 1536 + 512 * i:1536 + 512 * (i + 1)].rearrange(
                "p (k m) -> p k m", k=2) for i in range(3)]
            w4 = wp[:, 3072:3200].rearrange("p (k m) -> p k m", k=2)
            rt = wf[:, 0:18].rearrange("p (k a) -> p k a", k=2)
            bl = wf[:, 18:24].rearrange("p (k l) -> p k l", k=2)
            b4 = wf[0:64, 24:25]

            # ---- stage-2 conv input, padded resident:
            # block(a,w) x 464; top rows on partitions 0..63 written by S1
            # epilogues; bottom (row-shifted) copy built by banded DMAs.
            # Only pad columns need zeroing: u-band u occupies cols
            # [68+66u, 132+66u); unwritten = [0,68) + 2-col inter-band gaps
            # + [396,464).
            # L4 PSUM comes from the shared hps ring (partitions 0:64)

            def l4_psum():
                ps = hps.tile([128, 512], f32, tag="hp")
                return ps[0:64]

            resp = rpool.tile([128, A * WLOC, BW2], bf16)

            def resp_memsets():
                nc.vector.memset(resp[0:64, 0:36, 0:68], 0.0)
                nc.gpsimd.memset(resp[0:64, 36:72, 0:68], 0.0)
                nc.gpsimd.memset(resp[0:64, :, 396:BW2], 0.0)
                gaps = resp[0:64, :, 132:396].rearrange(
                    "p b (u c) -> p b u c", c=66)[:, :, :, 0:2]
                nc.vector.memset(gaps, 0.0)
            def hidden_layers(h1, ncols, mid_cb=None):
                # 512-col chunks (PSUM bank limit), a-block aligned.
                # k-split emission: all k=0 matmuls of a mt first, then
                # k=1 + epilogue -- extra shadow for the producer
                # epilogues of h_prev[:, 1, *].
                bounds = [(c0, min(c0 + 512, ncols))
                          for c0 in range(0, ncols, 512)]
                hprev = h1
                for li in range(3):
                    hn = hpool.tile([128, 2, ncols], bf16, tag="h")
                    for mt in range(2):
                        pss = []
                        for c0, c1 in bounds:
                            ps = hps.tile([128, 512], f32, tag="hp")
                            nc.tensor.matmul(
                                ps[:, 0:c1 - c0],
                                (wl[li][:, 0, mt * 128:(mt + 1) * 128]),
                                (hprev[:, 0, c0:c1]),
                                start=True, stop=False)
                            pss.append(ps)
                        for (c0, c1), ps in zip(bounds, pss):
                            nc.tensor.matmul(
                                ps[:, 0:c1 - c0],
                                (wl[li][:, 1, mt * 128:(mt + 1) * 128]),
                                (hprev[:, 1, c0:c1]),
                                start=False, stop=True)
                            epilogue(
                                hn[:, mt, c0:c1],
                                ps[:, 0:c1 - c0], bl[:, mt, li:li + 1], True)
                    if li == 0 and mid_cb is not None:
                        mid_cb()
                        mid_cb = None
                    hprev = hn
                return hprev

            # ================= STAGE 1 =================
            respv = resp[0:64, :, :].rearrange(
                "p (a w) x -> p a w x", a=A)

            def l4_stage1(h4, u, hg):
                # L4 -> scatter top rows into RESP (a-aligned chunks)
                base = 68 + 66 * u + hg * E1
                for a0, a1 in ((0, 4), (4, 8), (8, 9)):
                    na = a1 - a0
                    ps = l4_psum()
                    for k in range(2):
                        nc.tensor.matmul(
                            ps[:, 0:na * 128], (w4[:, k, :]),
                            (h4[:, k, a0 * 128:a1 * 128]),
                            start=(k == 0), stop=(k == 1))
                    dst = respv[:, a0:a1, :, base:base + E1].transpose(
                        (0, 1, 3, 2))
                    psv = ps[:, 0:na * 128].rearrange(
                        "p (aa e w) -> p aa e w", aa=na, e=E1)
                    epilogue(dst, psv, b4[:, 0:1], False)

            def band_dma(u, split=False):
                # u-band of resp top complete: stream its row-shifted
                # bottom copy. bot[:, c] = top[:, c+66].
                c0 = 66 * u
                blk_bounds = ((0, 8), (8, 72)) if split else ((0, 72),)
                for b0, b1 in blk_bounds:
                    nc.gpsimd.dma_start(
                        resp[64:128, b0:b1, c0:c0 + 66],
                        resp[0:64, b0:b1, c0 + 66:c0 + 132])

            pend = None
            for b in range(NB1):
                u, hg = b // 4, b % 4
                xin = xpool.tile([128, E1, BW1], bf16, tag="xin")
                nc.gpsimd.dma_start(
                    xin[:].rearrange("p e w -> p (e w)"),
                    xs_d[b].rearrange("p e w -> p (e w)"))
                if b == 0:
                    nc.sync.dma_start(wf[:], wf_d[:])
                    resp_memsets()
                if b == 1:
                    # tail of the bottom copy: only needs the zeroed pad;
                    # issued here to keep it off the t=0 critical path
                    nc.sync.dma_start(resp[64:128, :, 330:BW2 - 66],
                                      resp[0:64, :, 396:BW2])

                h1 = hpool.tile([128, 2, E1 * A * WLOC], bf16, tag="h")
                for mt in range(2):
                    zt = zpool.tile([128, 2, 512], f32, tag="z")
                    for bank in range(2):
                        es = bank * 8
                        zv = zt[:, bank, 0:8 * NPOS1]
                        for dj in range(3):
                            nc.tensor.matmul(
                                zv, (wa[:, dj, mt * 128:(mt + 1) * 128]),
                                (xin[:, es:es + 8, dj:dj + NPOS1]),
                                start=(dj == 0), stop=False)
                        for dj in range(3):
                            nc.tensor.matmul(
                                zv, (wb[64:128, dj, mt * 128:(mt + 1) * 128]),
                                (xin[64:128, es:es + 8, 10 + dj:10 + dj + NPOS1]),
                                start=False, stop=(dj == 2))
                    zg = zt[:, :, 0:8 * NPOS1].rearrange(
                        "p b (e v c) -> p b e v c", e=8, v=U)
                    # h1 cols a-major: each a-op writes one contiguous
                    # 128-col block -> downstream 512-chunks depend on
                    # exactly 4 a-ops
                    h1v = h1[:, mt, :].rearrange(
                        "p (a bk e w) -> p a bk e w", a=A, bk=2, e=8)
                    for a in range(A):
                        epilogue(h1v[:, a],
                                 zg[:, :, :, IY[a], 1:1 + WLOC],
                                 rt[:, mt, a:a + 1], True)
                if pend is not None:
                    # software-pipelined L4 of the previous batch: runs
                    # in the shadow after this batch's conv
                    l4_stage1(*pend)
                    if pend[2] == 3:
                        band_dma(pend[1])
                    pend = None

                h4 = hidden_layers(h1, E1 * A * WLOC)
                pend = (h4, u, hg)

            l4_stage1(*pend)
            band_dma(4, split=True)

            # ================= STAGE 2 =================
            def l4_stage2(h4, g):
                stg = opool.tile([64, E2 * A * HW], f32, tag="ostg")
                for c0 in range(0, E2 * A * HW, 512):
                    c1 = min(c0 + 512, E2 * A * HW)
                    ps = l4_psum()
                    for k in range(2):
                        nc.tensor.matmul(
                            ps[:, 0:c1 - c0], (w4[:, k, :]),
                            (h4[:, k, c0:c1]),
                            start=(k == 0), stop=(k == 1))
                    epilogue(stg[:, c0:c1], ps[:, 0:c1 - c0],
                             b4[:, 0:1], False)
                nc.gpsimd.dma_start(
                    out_d[:, :, 2 * g:2 * g + 2, :],
                    stg[:].rearrange("p (a b h) -> p a b h", a=A, b=E2))

            pend = None
            for g in range(NB2):
                h1 = hpool.tile([128, 2, E2 * A * HW], bf16, tag="h")
                for mt in range(2):
                    zt = zpool.tile([128, 2, 512], f32, tag="z")
                    for blk in range(2):
                        zv = zt[:, blk, 0:NPOS2]
                        for dj in range(3):
                            nc.tensor.matmul(
                                zv, (wa[:, dj, mt * 128:(mt + 1) * 128]),
                                (resp[:, 2 * g + blk, dj:dj + NPOS2]),
                                start=(dj == 0), stop=False)
                        for dj in range(3):
                            nc.tensor.matmul(
                                zv, (wb[64:128, dj, mt * 128:(mt + 1) * 128]),
                                (resp[64:128, 2 * g + blk,
                                      66 + dj:66 + dj + NPOS2]),
                                start=False, stop=(dj == 2))
                    h1v = h1[:, mt, :].rearrange(
                        "p (a bk h) -> p a bk h", a=A, bk=E2)
                    for a in range(A):
                        epilogue(h1v[:, a],
                                 zt[:, :, 1 + 66 * IY[a]:1 + 66 * IY[a] + HW],
                                 rt[:, mt, a:a + 1], True)
                if pend is not None:
                    l4_stage2(*pend)
                    pend = None

                h4 = hidden_layers(h1, E2 * A * HW)
                pend = (h4, g)

            l4_stage2(*pend)
        rep_ctx.close()
    nc.compile()
    return nc


def get_nc(repeat=1):
    key = f"nc{repeat}"
    if key not in _CACHE:
        _CACHE[key] = _build_nc(repeat)
    return _CACHE[key]


def host_prep(x, ws, bs):
    """Returns in_maps: list of 8 dicts of numpy arrays."""
    import ml_dtypes
    bfdt = ml_dtypes.bfloat16
    f = np.float32
    W0 = np.asarray(ws[0], f)
    W0r = np.ascontiguousarray(W0[:576].reshape(C, 3, 3, 256))   # [c, di, dj, m]
    wa = np.zeros((128, 3, 256), f)
    wa[:64] = W0r[:, 0]
    wa[64:] = W0r[:, 1]
    wb = np.zeros((128, 3, 256), f)
    wb[64:] = W0r[:, 2]
    rel0 = _rel0()
    R = np.asarray(bs[0], f)[None, :] + rel0[:, None] * W0[576]   # [9, 256]
    rt = np.ascontiguousarray(R.T.reshape(2, 128, A).transpose(1, 0, 2))
    wlt = [np.ascontiguousarray(np.asarray(ws[l], f).reshape(2, 128, 256)
                                .transpose(1, 0, 2)) for l in (1, 2, 3)]
    w4t = np.ascontiguousarray(np.asarray(ws[4], f).reshape(2, 128, 64)
                               .transpose(1, 0, 2))
    blt = np.ascontiguousarray(
        np.stack([np.asarray(bs[l], f).reshape(2, 128) for l in (1, 2, 3)],
                 axis=-1).transpose(1, 0, 2))                     # [128, 2, 3]
    b4t = np.ascontiguousarray(np.asarray(bs[4], f).reshape(64, 1))
    wp = np.zeros((128, WPCOLS), f)
    wp[:, 0:768] = wa.reshape(128, 768)
    wp[:, 768:1536] = wb.reshape(128, 768)
    for i in range(3):
        wp[:, 1536 + 512 * i:1536 + 512 * (i + 1)] = wlt[i].reshape(128, 512)
    wp[:, 3072:3200] = w4t.reshape(128, 128)
    wp = wp.astype(bfdt)
    wf = np.zeros((128, 25), f)
    wf[:, 0:18] = rt.reshape(128, 18)
    wf[:, 18:24] = blt.reshape(128, 6)
    wf[0:64, 24] = b4t[:, 0]

    x0 = np.asarray(x, f)[0]                                      # [C, U, V, H, W]
    xp = np.zeros((C, U, U, HW, HW + 2), f)
    xp[..., 1:HW + 1] = x0
    in_maps = []
    for k in range(NCORES):
        win = xp[..., 8 * k:8 * k + 10]                           # [C,U,V,H,10]
        top = np.zeros((C, U, HW, 7, 10), f)
        top[:, :, :, 1:6, :] = win.transpose(0, 1, 3, 2, 4)       # [C,U,H,V,10]
        blk = np.zeros((C, U, HW, BW1), f)
        blk[..., 1:71] = top.reshape(C, U, HW, 70)
        bot = np.zeros_like(blk)
        bot[..., 0:62] = blk[..., 10:72]
        full = np.concatenate([blk, bot], axis=0)                 # [128,U,H,72]
        xs = np.ascontiguousarray(
            full.transpose(1, 2, 0, 3).reshape(U, 4, E1, 128, BW1)
            .transpose(0, 1, 3, 2, 4).reshape(NB1, 128, E1, BW1))
        in_maps.append({"xs": xs.astype(bfdt), "wp": wp, "wf": wf})
    return in_maps


def assemble(results):
    out = np.empty((1, C, A, A, HW, HW), np.float32)
    for k in range(NCORES):
        ok = np.asarray(results[k]["out"]).reshape(C, A, A, WLOC, HW)
        out[0, :, :, :, :, 8 * k:8 * k + 8] = ok.transpose(0, 1, 2, 4, 3)
    return out


def run(in_maps, trace=False, repeat=1, **kw):
    from concourse.bass_utils import run_bass_kernel_spmd
    return run_bass_kernel_spmd(get_nc(repeat), in_maps, list(range(NCORES)),
                                trace=trace, **kw)


def kernel(x, w0, b0, w1, b1, w2, b2, w3, b3, w4, b4, patchsize=64, ang_factor=9):
    ws = [w0, w1, w2, w3, w4]
    bs = [b0, b1, b2, b3, b4]
    in_maps = host_prep(x, ws, bs)
    res = run(in_maps)
    return assemble(res.results)

